# revision 1
# baseline (speedup 1.0000x reference)
"""Trainium2 Bass kernel: causal self-attention with RoPE (B=4, T=2048, D=1024, H=16, Dh=64).

Sharding: 8 cores = 4 batches x 2 head-halves. Core c handles batch c//2 and
heads (c%2)*8 .. (c%2)*8+7 (feature columns (c%2)*512 .. +512 of Wq/Wk/Wv, and
the matching rows of Wo). Each core computes a partial output [T, D]; the host
sums the two partials per batch (row-sharded Wo reduction) and stacks batches.

On-chip layout: activations are kept transposed (features on partitions):
  xT [D, T] (spilled to DRAM), qT/kT [512, T], scoresT [s, t], attn_outT [512, T].
This makes every matmul contraction land on the partition dim with zero
transposes except one PE-transpose pass over x. The softmax denominator is
fused into the AV matmul via a ones-column appended to V (M=65), and the
causal mask is applied post-exp with a single tensor_mask per diagonal group.
"""

import os
import sys

for _p in ("/opt/trn_rl_repo", "/root/.axon_site/_ro/trn_rl_repo"):
    if os.path.isdir(_p) and _p not in sys.path:
        sys.path.append(_p)

import numpy as np

import bass_rust
import concourse.bass as bass
import concourse.mybir as mybir
import concourse.tile as tile
from concourse.vector_clock import ScopedClock

F32 = mybir.dt.float32
F32R = mybir.dt.float32r
BF16 = mybir.dt.bfloat16

B, T, D, H, Dh = 4, 2048, 1024, 16, 64
FC = 512          # features per core (8 heads)
NG = 2            # head groups per core (4 heads each)
FG = FC // NG     # 256 features per group
NTC = T // 512    # 4 t-chunks
NTT = T // 128    # 16 t-tiles
ND = D // 128     # 8 d-chunks


class _TC(tile.TileContext):
    """TileContext whose tail Drain carries at most one sem wait.

    The walrus build in this container rejects a Drain with >1 sync waits
    (setupSyncWait: "Too many sync wait commands"), so spread the waits over
    a chain of Drain instructions instead.
    """

    def _drain_and_barrier(self, tick_clock, wait_clock):
        drain_inst = self.nc.sync.drain()
        wait_clock.add_sem_waits(
            drain_inst.ins, ScopedClock({None: tick_clock.global_clock})
        )
        si = drain_inst.ins.sync_info
        if si is not None and len(si.on_wait) > 1:
            waits = list(si.on_wait)
            drain_inst.ins.sync_info = bass_rust.SyncInfo(
                on_wait=waits[:1], on_update=list(si.on_update)
            )
            for w in waits[1:]:
                d2 = self.nc.sync.drain()
                d2.ins.sync_info = bass_rust.SyncInfo(on_wait=[w], on_update=[])
        self.nc.all_engine_barrier()
        popped = self.nc._tile_sem_poison_stack.pop()
        assert popped is self._sem_poison
        self.nc.clear_and_free_semaphores(list(self.sems.allocated().values()))
        self.nc.all_engine_barrier()


def _r(ap):
    return ap.bitcast(F32R)


def _split_waits(nc, max_waits=1):
    """Hoist extra sem waits onto same-engine NoOps.

    The walrus build here allows only one sync wait on several instruction
    structs (Drain, the fp32/fp32r matmul LW struct). Engine queues are
    in-order, so moving waits to a preceding NoOp on the same engine is
    semantics-preserving.
    """
    n = 0
    for fn in nc.m.functions:
        for bb in fn.blocks:
            out = []
            for inst in bb.instructions:
                si = inst.sync_info
                if si is not None and len(si.on_wait) > max_waits:
                    waits = list(si.on_wait)
                    extra, keep = waits[:-max_waits], waits[-max_waits:]
                    for i, w in enumerate(extra):
                        nop = mybir.InstNoOp(
                            name=f"{inst.name}_ws{i}", engine=inst.engine
                        )
                        nop.sync_info = bass_rust.SyncInfo(on_wait=[w], on_update=[])
                        out.append(nop)
                        n += 1
                    inst.sync_info = bass_rust.SyncInfo(
                        on_wait=keep, on_update=list(si.on_update)
                    )
                out.append(inst)
            bb.instructions = out
    return n


def _build_program():
    from contextlib import ExitStack

    nc = bass.Bass()

    x = nc.dram_tensor("x", [T, D], F32, kind="ExternalInput")
    wq = nc.dram_tensor("wq", [D, FC], F32R, kind="ExternalInput")
    wk = nc.dram_tensor("wk", [D, FC], F32R, kind="ExternalInput")
    wv = nc.dram_tensor("wv", [D, FC], F32R, kind="ExternalInput")
    wo = nc.dram_tensor("wo", [FC, D], F32R, kind="ExternalInput")
    cos2 = nc.dram_tensor("cos2", [128, T], F32, kind="ExternalInput")
    sin2 = nc.dram_tensor("sin2", [128, T], F32, kind="ExternalInput")
    ident = nc.dram_tensor("ident", [128, 128], F32, kind="ExternalInput")
    mk0 = nc.dram_tensor("mk0", [128, 1024], F32, kind="ExternalInput")
    mk256 = nc.dram_tensor("mk256", [128, 1024], F32, kind="ExternalInput")
    ones8 = nc.dram_tensor("ones8", [128, 8], F32R, kind="ExternalInput")
    ones64 = nc.dram_tensor("ones64", [1, 64], F32R, kind="ExternalInput")
    out = nc.dram_tensor("out", [T, D], F32, kind="ExternalOutput")

    with _TC(nc) as tc, ExitStack() as ctx:
        consts = ctx.enter_context(tc.tile_pool(name="consts", bufs=1))
        # PSUM: 2x [128,1024] double-bank slots + 4x [128,512] single-bank slots
        psum = ctx.enter_context(tc.tile_pool(name="psum", bufs=2, space="PSUM"))
        psums = ctx.enter_context(tc.tile_pool(name="psums", bufs=4, space="PSUM"))
        dram = ctx.enter_context(tc.tile_pool(name="dram", bufs=4, space="DRAM"))
        persist = ctx.enter_context(tc.tile_pool(name="persist", bufs=1))
        wp = ctx.enter_context(tc.tile_pool(name="wp", bufs=1))

        ident_t = consts.tile([128, 128], F32)
        nc.sync.dma_start(ident_t[:], ident[:])
        ones64_t = consts.tile([1, 64], F32R)
        nc.sync.dma_start(ones64_t[:], ones64[:])
        mk0_t = consts.tile([128, 1024], F32)
        nc.sync.dma_start(mk0_t[:], mk0[:])
        mk256_t = consts.tile([128, 1024], F32)
        nc.sync.dma_start(mk256_t[:], mk256[:])

        def load_weights(g):
            gsl = slice(g * FG, (g + 1) * FG)
            tiles = []
            for nm, wsrc in (("wq", wq), ("wk", wk)):
                w_t = wp.tile([128, ND * FG], F32R, tag=nm, name=f"{nm}_t{g}")
                nc.sync.dma_start(
                    w_t[:].rearrange("p (d f) -> p d f", d=ND),
                    wsrc[:, gsl].rearrange("(d p) f -> p d f", p=128),
                )
                tiles.append(w_t)
            return tiles

        wv_t = wp.tile([128, ND * FC], F32R, tag="wv", name="wv_t")
        nc.sync.dma_start(
            wv_t[:].rearrange("p (d f) -> p d f", d=ND),
            wv[:].rearrange("(d p) f -> p d f", p=128),
        )

        # attention outputs, persistent across both groups: 4 f-chunks [128, T]
        ao = [persist.tile([128, T], F32R, tag=f"ao{i}", name=f"ao{i}") for i in range(4)]
        # v (natural layout) with a ones column per head: 8 heads x 65 cols
        vt = [persist.tile([128, 8 * 65], F32R, tag=f"vt{i}", name=f"vt{i}") for i in range(NTT)]

        wtiles = load_weights(0)
        for i in range(NTT):
            nc.sync.dma_start(vt[i][:, 64::65], ones8[:])
        for g in range(NG):
            # per-group persistent activations (slots reused across groups)
            qt = [persist.tile([128, T], BF16, tag=f"qt{i}", name=f"qt{i}g{g}") for i in range(2)]
            kt = [persist.tile([128, T], BF16, tag=f"kt{i}", name=f"kt{i}g{g}") for i in range(2)]

            with ExitStack() as gctx:
                # ---- Phase B(g): projections + RoPE
                xtc = gctx.enter_context(tc.tile_pool(name=f"xtc{g}", bufs=3))
                rop = gctx.enter_context(tc.tile_pool(name=f"rope{g}", bufs=3))
                xld = gctx.enter_context(tc.tile_pool(name=f"xload{g}", bufs=4))

                wq_t, wk_t = wtiles

                for tcc in range(NTC):
                    csl = slice(tcc * 512, (tcc + 1) * 512)
                    cos_t = rop.tile([128, 512], F32, tag="cs", name="cos_t", bufs=2)
                    nc.sync.dma_start(cos_t[:], cos2[:, csl])
                    sin_t = rop.tile([128, 512], F32, tag="sn", name="sin_t", bufs=2)
                    nc.sync.dma_start(sin_t[:], sin2[:, csl])
                    # rotate_half sign fold: rows 0:32 / 64:96 get -sin
                    nc.scalar.mul(sin_t[0:32, :], sin_t[0:32, :], -1.0)
                    nc.scalar.mul(sin_t[64:96, :], sin_t[64:96, :], -1.0)
                    xc = xtc.tile([128, ND * 512], F32R, tag="xc")
                    # transpose x[tc] on the fly, one x-tile at a time
                    xcv = xc[:].rearrange("p (d t) -> p d t", d=ND)
                    for q in range(4):
                        t0 = (tcc * 4 + q) * 128
                        xt_ = xld.tile([128, D], F32, tag="xl")
                        nc.sync.dma_start(xt_[:], x[t0 : t0 + 128, :])
                        for dh in range(2):
                            tp = psums.tile([128, 512], F32, tag="sm", name="tp")
                            for dl in range(4):
                                d = dh * 4 + dl
                                nc.tensor.transpose(
                                    tp[:, dl * 128 : (dl + 1) * 128],
                                    xt_[:, d * 128 : (d + 1) * 128],
                                    ident_t[:],
                                )
                            nc.vector.tensor_copy(
                                xcv[:, dh * 4 : dh * 4 + 4, q * 128 : (q + 1) * 128],
                                tp[:].rearrange("p (dl t) -> p dl t", dl=4),
                            )
                    tsl = slice(tcc * 512, (tcc + 1) * 512)
                    # q/k projections (transposed outputs) + RoPE
                    for dst, w_t in ((qt, wq_t), (kt, wk_t)):
                        ps = psum.tile([128, 1024], F32, tag="pp")
                        for fp in range(2):
                            for d in range(ND):
                                nc.tensor.matmul(
                                    ps[:, fp * 512 : fp * 512 + 512],
                                    w_t[:, d * FG + fp * 128 : d * FG + (fp + 1) * 128],
                                    xc[:, d * 512 : (d + 1) * 512],
                                    start=(d == 0),
                                    stop=(d == ND - 1),
                                )
                        for fp in range(2):
                            psl = ps[:, fp * 512 : fp * 512 + 512]
                            raw = rop.tile([128, 512], F32, tag="raw")
                            nc.scalar.copy(raw[:], psl)
                            rot = rop.tile([128, 512], F32, tag="rot")
                            for hb in range(2):
                                o = hb * 64
                                nc.sync.dma_start(rot[o : o + 32, :], raw[o + 32 : o + 64, :])
                                nc.sync.dma_start(rot[o + 32 : o + 64, :], raw[o : o + 32, :])
                            dtile = dst[fp]
                            nc.vector.tensor_mul(dtile[:, tsl], psl, cos_t[:])
                            nc.vector.tensor_mul(rot[:], rot[:], sin_t[:])
                            nc.vector.tensor_add(dtile[:, tsl], dtile[:, tsl], rot[:])
                    # v projection for all 8 heads at once (group 0 only)
                    if g == 0:
                        for tb in range(4):
                            pv = psums.tile([128, 512], F32, tag="sm", name="pv")
                            for d in range(ND):
                                nc.tensor.matmul(
                                    pv[:],
                                    xc[:, d * 512 + tb * 128 : d * 512 + (tb + 1) * 128],
                                    wv_t[:, d * FC : (d + 1) * FC],
                                    start=(d == 0),
                                    stop=(d == ND - 1),
                                )
                            i = tcc * 4 + tb
                            vdst = vt[i][:, :].rearrange("p (h c) -> p h c", c=65)[
                                :, :, 0:64
                            ]
                            vsrc = pv[:].rearrange("p (h c) -> p h c", c=64)
                            nc.vector.tensor_copy(vdst, vsrc)

                if g == 0:
                    # prefetch group-1 weights while attention of group 0 runs
                    wtiles = load_weights(1)

            # ---- Phase C(g): attention. The pair's two heads are interleaved
            # so one head's QK fills the PE while the other waits on exp.
            with ExitStack() as cctx:
                ep = cctx.enter_context(tc.tile_pool(name=f"exp{g}", bufs=4))
                rp = cctx.enter_context(tc.tile_pool(name=f"rcp{g}", bufs=3))
                for tcc in (3, 2, 1, 0):
                    tsl = slice(tcc * 512, (tcc + 1) * 512)
                    ngrp = 2 * tcc + 2
                    for fp in range(2):
                        avs = []
                        for ho in range(2):
                            av_ps = psums.tile(
                                [128, 512], F32, tag="sm", name=f"av{ho}"
                            )
                            avs.append(av_ps)
                        for g2 in range(ngrp):
                            exs = []
                            for ho in range(2):
                                o = ho * 64
                                sc = psum.tile([128, 1024], F32, tag="pp", name="sc")
                                for half in range(2):
                                    si = 2 * g2 + half
                                    nc.tensor.matmul(
                                        sc[:, half * 512 : half * 512 + 512],
                                        kt[fp][o : o + 64, si * 128 : (si + 1) * 128],
                                        qt[fp][o : o + 64, tsl],
                                        start=True,
                                        stop=True,
                                    )
                                ex = ep.tile([128, 1024], F32R, tag="ex")
                                nc.scalar.activation(
                                    ex[:], sc[:], mybir.ActivationFunctionType.Exp,
                                    scale=0.125,
                                )
                                if g2 >= 2 * tcc:
                                    mt = mk0_t if g2 == 2 * tcc else mk256_t
                                    nc.vector.tensor_mul(ex[:], ex[:], mt[:])
                                exs.append(ex)
                            for ho in range(2):
                                hl = 2 * fp + ho
                                for half in range(2):
                                    si = 2 * g2 + half
                                    nc.tensor.matmul(
                                        avs[ho][0:65, :],
                                        vt[si][:, (4 * g + hl) * 65 : (4 * g + hl) * 65 + 65],
                                        exs[ho][:, half * 512 : half * 512 + 512],
                                        start=(g2 == 0 and half == 0),
                                        stop=(g2 == ngrp - 1 and half == 1),
                                    )
                        for ho in range(2):
                            o = ho * 64
                            av_sb = rp.tile([65, 512], F32, tag="avs")
                            nc.vector.tensor_copy(av_sb[:], avs[ho][0:65, :])
                            rcp = rp.tile([1, 512], F32R, tag="rc")
                            with nc.allow_low_precision(reason="f32r recip"):
                                nc.vector.reciprocal(rcp[:], av_sb[64:65, :])
                            pb = psums.tile([128, 512], F32, tag="sm", name="pb")
                            nc.tensor.matmul(
                                pb[0:64, :], ones64_t[:], rcp[:], start=True, stop=True
                            )
                            nc.vector.tensor_mul(
                                ao[2 * g + fp][o : o + 64, tsl],
                                av_sb[0:64, :],
                                pb[0:64, :],
                            )

        # ---- Phase D: output projection (row-sharded Wo partial)
        with tc.tile_pool(name="wo", bufs=1) as wop, tc.tile_pool(
            name="oev", bufs=3
        ) as oev:
            wo_t = wop.tile([128, 4 * D], F32R, tag="wo")
            nc.sync.dma_start(
                wo_t[:].rearrange("p (c o) -> p c o", c=4),
                wo[:].rearrange("(c p) o -> p c o", p=128),
            )
            for i in range(NTT):
                po = psum.tile([128, 1024], F32, tag="pp", name="po")
                for n in range(2):
                    for c in range(4):
                        nc.tensor.matmul(
                            po[:, n * 512 : n * 512 + 512],
                            ao[c][:, i * 128 : (i + 1) * 128],
                            wo_t[:, c * D + n * 512 : c * D + n * 512 + 512],
                            start=(c == 0),
                            stop=(c == 3),
                        )
                oe = oev.tile([128, 1024], F32, tag="oe")
                nc.scalar.copy(oe[:], po[:])
                nc.sync.dma_start(out[i * 128 : (i + 1) * 128, :], oe[:])

    _split_waits(nc)
    return nc

_NC_CACHE = None


def _get_nc():
    global _NC_CACHE
    if _NC_CACHE is None:
        _NC_CACHE = _build_program()
    return _NC_CACHE


def _consts():
    j = np.arange(1024)
    p = np.arange(128)
    s_rel = p[:, None] + 128 * (j[None, :] // 512)  # s offset within group
    t_rel = j[None, :] % 512
    return {
        "ident": np.eye(128, dtype=np.float32),
        "mk0": (s_rel <= t_rel).astype(np.float32),
        "mk256": (s_rel + 256 <= t_rel).astype(np.float32),
        "ones8": np.ones((128, 8), dtype=np.float32),
        "ones64": np.ones((1, 64), dtype=np.float32),
    }


def kernel(x, cos, sin, Wq, Wk, Wv, Wo):
    from concourse.bass_utils import run_bass_kernel_spmd

    x = np.asarray(x, dtype=np.float32)
    cos = np.asarray(cos, dtype=np.float32)
    sin = np.asarray(sin, dtype=np.float32)
    Wq = np.asarray(Wq, dtype=np.float32)
    Wk = np.asarray(Wk, dtype=np.float32)
    Wv = np.asarray(Wv, dtype=np.float32)
    Wo = np.asarray(Wo, dtype=np.float32)

    cos2 = np.ascontiguousarray(np.tile(cos.T, (2, 1)))  # [128, T]
    sin2 = np.ascontiguousarray(np.tile(sin.T, (2, 1)))
    consts = _consts()

    in_maps = []
    for c in range(8):
        b, hh = c // 2, c % 2
        sl = slice(hh * FC, (hh + 1) * FC)
        in_maps.append(
            {
                "x": np.ascontiguousarray(x[b]),
                "wq": np.ascontiguousarray(Wq[:, sl]),
                "wk": np.ascontiguousarray(Wk[:, sl]),
                "wv": np.ascontiguousarray(Wv[:, sl]),
                "wo": np.ascontiguousarray(Wo[sl, :]),
                "cos2": cos2,
                "sin2": sin2,
                **consts,
            }
        )

    nc = _get_nc()
    res = run_bass_kernel_spmd(nc, in_maps, core_ids=list(range(8)))
    outs = [res.results[c]["out"] for c in range(8)]
    full = np.stack([outs[2 * b] + outs[2 * b + 1] for b in range(B)])
    return full.astype(np.float32)



# revision 5
# speedup vs baseline: 1.3435x; 1.3435x over previous
"""Trainium2 Bass kernel: causal self-attention with RoPE (B=4, T=2048, D=1024, H=16, Dh=64).

Sharding: 8 cores = 4 batches x 2 head-halves. Core c handles batch c//2 and
heads (c%2)*8 .. +8 (feature columns (c%2)*512 of Wq/Wk/Wv, matching rows of
Wo). Each core computes a partial [T, D] output; the host sums the two
partials per batch.

v2 layout: x is transposed ON THE HOST to xT [D, T] bf16 (no on-chip
transposes). All matmul inputs are bf16. Single pass over all 8 heads:
  per t-chunk (512): q/k projections (transposed out [f, t]) + RoPE, v
  projection (natural [t, f] with a fused ones column per head), then
  causal attention for that chunk interleaved with the next chunk's
  projections. Scores matmuls are K=64 row-pairs (two heads concurrent in
  the PE array); AV uses M=65 (denominator fused via the ones column).
  Diagonal s-blocks narrow the exp/AV width to the causal region and apply
  a single [128,128] triangle mask. exp is the only scalar-engine work.
"""

import os
import sys

for _p in ("/opt/trn_rl_repo", "/root/.axon_site/_ro/trn_rl_repo"):
    if os.path.isdir(_p) and _p not in sys.path:
        sys.path.append(_p)

import numpy as np

import bass_rust
import concourse.bass as bass
import concourse.mybir as mybir
import concourse.tile as tile
from concourse.vector_clock import ScopedClock

F32 = mybir.dt.float32
F32R = mybir.dt.float32r
BF16 = mybir.dt.bfloat16

B, T, D, H, Dh = 4, 2048, 1024, 16, 64
FC = 512          # features per core (8 heads)
NF = 4            # head-pair tiles (128 features each)
NTC = T // 512    # 4 t-chunks
NTT = T // 128    # 16 t-tiles
ND = D // 128     # 8 d-chunks


class _TC(tile.TileContext):
    """TileContext whose tail Drain carries at most one sem wait."""

    def _drain_and_barrier(self, tick_clock, wait_clock):
        drain_inst = self.nc.sync.drain()
        wait_clock.add_sem_waits(
            drain_inst.ins, ScopedClock({None: tick_clock.global_clock})
        )
        si = drain_inst.ins.sync_info
        if si is not None and len(si.on_wait) > 1:
            waits = list(si.on_wait)
            drain_inst.ins.sync_info = bass_rust.SyncInfo(
                on_wait=waits[:1], on_update=list(si.on_update)
            )
            for w in waits[1:]:
                d2 = self.nc.sync.drain()
                d2.ins.sync_info = bass_rust.SyncInfo(on_wait=[w], on_update=[])
        self.nc.all_engine_barrier()
        popped = self.nc._tile_sem_poison_stack.pop()
        assert popped is self._sem_poison
        self.nc.clear_and_free_semaphores(list(self.sems.allocated().values()))
        self.nc.all_engine_barrier()


def _r(ap):
    return ap.bitcast(F32R)


def _split_waits(nc, max_waits=1):
    """Hoist extra sem waits onto same-engine NoOps (walrus 1-wait limit)."""
    n = 0
    for fn in nc.m.functions:
        for bb in fn.blocks:
            out = []
            for inst in bb.instructions:
                si = inst.sync_info
                if si is not None and len(si.on_wait) > max_waits:
                    waits = list(si.on_wait)
                    extra, keep = waits[:-max_waits], waits[-max_waits:]
                    for i, w in enumerate(extra):
                        nop = mybir.InstNoOp(
                            name=f"{inst.name}_ws{i}", engine=inst.engine
                        )
                        nop.sync_info = bass_rust.SyncInfo(on_wait=[w], on_update=[])
                        out.append(nop)
                        n += 1
                    inst.sync_info = bass_rust.SyncInfo(
                        on_wait=keep, on_update=list(si.on_update)
                    )
                out.append(inst)
            bb.instructions = out
    return n


def _build_program():
    from contextlib import ExitStack

    nc = bass.Bass()

    xt = nc.dram_tensor("xt", [D, T], BF16, kind="ExternalInput")
    wq = nc.dram_tensor("wq", [D, FC], BF16, kind="ExternalInput")
    wk = nc.dram_tensor("wk", [D, FC], BF16, kind="ExternalInput")
    wv = nc.dram_tensor("wv", [D, FC], BF16, kind="ExternalInput")
    wo = nc.dram_tensor("wo", [FC, D], BF16, kind="ExternalInput")
    cos2 = nc.dram_tensor("cos2", [128, T], BF16, kind="ExternalInput")
    sin2 = nc.dram_tensor("sin2", [128, T], BF16, kind="ExternalInput")
    tri = nc.dram_tensor("tri", [128, 128], BF16, kind="ExternalInput")
    ones8 = nc.dram_tensor("ones8", [128, 8], BF16, kind="ExternalInput")
    ones64 = nc.dram_tensor("ones64", [1, 64], F32R, kind="ExternalInput")
    out = nc.dram_tensor("out", [T, D], F32, kind="ExternalOutput")

    with _TC(nc) as tc, ExitStack() as ctx:
        consts = ctx.enter_context(tc.tile_pool(name="consts", bufs=1))
        persist = ctx.enter_context(tc.tile_pool(name="persist", bufs=1))
        wp = ctx.enter_context(tc.tile_pool(name="wp", bufs=1))
        # PSUM: sc 2x[128,1024] (4 banks) + ps 2x[128,512] + av 2x[128,512]
        scp = ctx.enter_context(tc.tile_pool(name="scp", bufs=2, space="PSUM"))
        psp = ctx.enter_context(tc.tile_pool(name="psp", bufs=2, space="PSUM"))
        avp = ctx.enter_context(tc.tile_pool(name="avp", bufs=2, space="PSUM"))
        # SBUF working pools
        xcp = ctx.enter_context(tc.tile_pool(name="xcp", bufs=2))
        rop = ctx.enter_context(tc.tile_pool(name="rop", bufs=2))
        exp_ = ctx.enter_context(tc.tile_pool(name="exp", bufs=4))
        nrm = ctx.enter_context(tc.tile_pool(name="nrm", bufs=3))
        oev = ctx.enter_context(tc.tile_pool(name="oev", bufs=3))

        # ---- constants / weights
        tri_t = consts.tile([128, 128], BF16)
        nc.sync.dma_start(tri_t[:], tri[:])
        ones64_t = consts.tile([1, 64], F32R)
        nc.sync.dma_start(ones64_t[:], ones64[:])
        cos_t = consts.tile([128, T], BF16)
        nc.sync.dma_start(cos_t[:], cos2[:])
        sin_t = consts.tile([128, T], BF16)
        nc.sync.dma_start(sin_t[:], sin2[:])

        wtiles = {}
        for nm, wsrc in (("wq", wq), ("wk", wk), ("wv", wv)):
            w_t = wp.tile([128, ND * FC], BF16, tag=nm, name=f"{nm}_t")
            nc.sync.dma_start(
                w_t[:].rearrange("p (d f) -> p d f", d=ND),
                wsrc[:].rearrange("(d p) f -> p d f", p=128),
            )
            wtiles[nm] = w_t
        wo_t = wp.tile([128, 4 * D], BF16, tag="wo", name="wo_t")
        nc.sync.dma_start(
            wo_t[:].rearrange("p (c o) -> p c o", c=4),
            wo[:].rearrange("(c p) o -> p c o", p=128),
        )

        # ---- persistent activations
        qt = [persist.tile([128, T], BF16, tag=f"qt{f}", name=f"qt{f}") for f in range(NF)]
        kt = [persist.tile([128, T], BF16, tag=f"kt{f}", name=f"kt{f}") for f in range(NF)]
        # v natural layout, 65 cols per head (ones col fused at index 64)
        vt = [persist.tile([128, 8 * 65], BF16, tag=f"vt{i}", name=f"vt{i}") for i in range(NTT)]
        ao = [persist.tile([128, T], BF16, tag=f"ao{f}", name=f"ao{f}") for f in range(NF)]
        for i in range(NTT):
            nc.sync.dma_start(vt[i][:, 64::65], ones8[:])

        xc = [None] * NTC

        def emit_load_x(tcc):
            xc[tcc] = xcp.tile([128, ND * 512], BF16, tag="xc", name=f"xc{tcc}")
            nc.sync.dma_start(
                xc[tcc][:].rearrange("p (d t) -> p d t", d=ND),
                xt[:, tcc * 512 : (tcc + 1) * 512].rearrange(
                    "(d p) t -> p d t", p=128
                ),
            )

        def emit_proj_unit(tcc, dst, f):
            """One q/k projection f-tile: 8 accum MMs + RoPE (returns nothing)."""
            tsl = slice(tcc * 512, (tcc + 1) * 512)
            w_t = wtiles["wq" if dst == 0 else "wk"]
            dtile = (qt if dst == 0 else kt)[f]
            ps = psp.tile([128, 512], F32, tag="ps", name=f"ps{tcc}d{dst}f{f}")
            for d in range(ND):
                nc.tensor.matmul(
                    ps[:],
                    w_t[:, d * FC + f * 128 : d * FC + (f + 1) * 128],
                    xc[tcc][:, d * 512 : (d + 1) * 512],
                    start=(d == 0),
                    stop=(d == ND - 1),
                )
            # RoPE: rawc = bf16(ps); rot = swap32(rawc); out = rawc*c + rot*s
            rawc = rop.tile([128, 512], BF16, tag="raw", name=f"raw{tcc}d{dst}f{f}")
            nc.vector.tensor_copy(rawc[:], ps[:])
            rot = rop.tile([128, 512], BF16, tag="rot", name=f"rot{tcc}d{dst}f{f}")
            for o in (0, 64):
                nc.sync.dma_start(rot[o : o + 32, :], rawc[o + 32 : o + 64, :])
                nc.sync.dma_start(rot[o + 32 : o + 64, :], rawc[o : o + 32, :])
            tmp = rop.tile([128, 512], BF16, tag="tmp", name=f"tmp{tcc}d{dst}f{f}")
            nc.vector.tensor_mul(tmp[:], rot[:], sin_t[:, tsl])
            nc.vector.tensor_mul(dtile[:, tsl], rawc[:], cos_t[:, tsl])
            nc.vector.tensor_add(dtile[:, tsl], dtile[:, tsl], tmp[:])

        def emit_v_unit(tcc, tb):
            """One v-projection t-tile (128 t rows, all 512 features)."""
            pv = psp.tile([128, 512], F32, tag="ps", name=f"pv{tcc}b{tb}")
            for d in range(ND):
                nc.tensor.matmul(
                    pv[:],
                    xc[tcc][:, d * 512 + tb * 128 : d * 512 + (tb + 1) * 128],
                    wtiles["wv"][:, d * FC : (d + 1) * FC],
                    start=(d == 0),
                    stop=(d == ND - 1),
                )
            i = tcc * 4 + tb
            vdst = vt[i][:, :].rearrange("p (h c) -> p h c", c=65)[:, :, 0:64]
            vsrc = pv[:].rearrange("p (h c) -> p h c", c=64)
            nc.vector.tensor_copy(vdst, vsrc)

        def proj_units(tcc):
            """Generator of emit-thunks for t-chunk tcc's projections."""
            yield lambda: emit_load_x(tcc)
            for dst in range(2):
                for f in range(NF):
                    yield lambda d=dst, ff=f: emit_proj_unit(tcc, d, ff)
            for tb in range(4):
                yield lambda b=tb: emit_v_unit(tcc, b)

        def attn_units(tcc):
            """Generator of emit-thunks for attention over t-chunk tcc."""
            ngrp = 2 * tcc + 2
            t0 = tcc * 512
            for fp in range(NF):
                def fp_block(fp=fp, ngrp=ngrp, t0=t0, tcc=tcc):
                    avs = []
                    for ho in range(2):
                        av = avp.tile([128, 512], F32, tag="av", name=f"av{tcc}p{fp}h{ho}")
                        avs.append(av)

                    def emit_scores(g2):
                        pair = [
                            scp.tile([128, 1024], F32, tag="sc", name=f"sc{tcc}p{fp}g{g2}h{ho}")
                            for ho in range(2)
                        ]
                        # half-outer order: (ho0,ho1) pairs hit disjoint PE
                        # row groups back-to-back and overlap in the array
                        for half in range(2):
                            si = 2 * g2 + half
                            j = si - 4 * tcc
                            col0 = j * 128 if j >= 0 else 0
                            for ho in range(2):
                                o = ho * 64
                                nc.tensor.matmul(
                                    pair[ho][:, half * 512 + col0 : (half + 1) * 512],
                                    kt[fp][o : o + 64, si * 128 : (si + 1) * 128],
                                    qt[fp][o : o + 64, t0 + col0 : t0 + 512],
                                    start=True,
                                    stop=True,
                                )
                        return pair

                    def emit_exp(g2, sc_pair):
                        diag_odd = g2 == 2 * tcc + 1
                        lo = 256 if diag_odd else 0
                        expair = []
                        for ho in range(2):
                            ex = exp_.tile([128, 1024], BF16, tag="ex", name=f"ex{tcc}p{fp}g{g2}h{ho}")
                            nc.scalar.activation(
                                ex[:, lo:1024],
                                sc_pair[ho][:, lo:1024],
                                mybir.ActivationFunctionType.Exp,
                                scale=0.125,
                            )
                            if g2 >= 2 * tcc:
                                for half in range(2):
                                    j = 2 * g2 + half - 4 * tcc
                                    c0 = half * 512 + j * 128
                                    nc.vector.tensor_mul(
                                        ex[:, c0 : c0 + 128],
                                        ex[:, c0 : c0 + 128],
                                        tri_t[:],
                                    )
                            expair.append(ex)
                        return expair

                    def emit_av(g2, expair):
                        for ho in range(2):
                            for half in range(2):
                                si = 2 * g2 + half
                                j = si - 4 * tcc
                                col0 = j * 128 if j >= 0 else 0
                                nc.tensor.matmul(
                                    avs[ho][0:65, col0:512],
                                    vt[si][:, (2 * fp + ho) * 65 : (2 * fp + ho) * 65 + 65],
                                    expair[ho][:, half * 512 + col0 : (half + 1) * 512],
                                    start=(g2 == 0 and half == 0),
                                    stop=(g2 == ngrp - 1 and half == 1),
                                )

                    # software pipeline: scores(g2) -> exp(g2) -> av(g2)
                    pend = {}
                    for g2 in range(ngrp):
                        sc_pair = emit_scores(g2)
                        pend[g2] = emit_exp(g2, sc_pair)
                        if g2 - 1 in pend:
                            emit_av(g2 - 1, pend.pop(g2 - 1))
                    emit_av(ngrp - 1, pend.pop(ngrp - 1))

                    # normalization: ao[rows ho*64..][tsl] = av[0:64]/av[64]
                    for ho in range(2):
                        avsb = nrm.tile([65, 512], F32, tag="avs", name=f"avs{tcc}p{fp}h{ho}")
                        nc.vector.tensor_copy(avsb[:], avs[ho][0:65, :])
                        rc = nrm.tile([1, 512], F32R, tag="rc", name=f"rc{tcc}p{fp}h{ho}")
                        with nc.allow_low_precision(reason="f32r recip"):
                            nc.vector.reciprocal(rc[:], _r(avsb[64:65, :]))
                        pb = psp.tile([128, 512], F32, tag="ps", name=f"pb{tcc}p{fp}h{ho}")
                        nc.tensor.matmul(
                            pb[0:64, :], ones64_t[:], rc[:], start=True, stop=True
                        )
                        nc.vector.tensor_mul(
                            ao[fp][ho * 64 : ho * 64 + 64, t0 : t0 + 512],
                            avsb[0:64, :],
                            pb[0:64, :],
                        )

                yield fp_block

        def out_units(tcc):
            """Output projection for the 4 t-tiles of chunk tcc."""
            for tb in range(4):
                def unit(tb=tb):
                    i = tcc * 4 + tb
                    oe = oev.tile([128, D], F32, tag="oe", name=f"oe{i}")
                    for n in range(2):
                        po = psp.tile([128, 512], F32, tag="ps", name=f"po{i}n{n}")
                        for c in range(4):
                            nc.tensor.matmul(
                                po[:],
                                ao[c][:, i * 128 : (i + 1) * 128],
                                wo_t[:, c * D + n * 512 : c * D + n * 512 + 512],
                                start=(c == 0),
                                stop=(c == 3),
                            )
                        eng = nc.vector if (i + n) % 2 == 0 else nc.scalar
                        if eng is nc.vector:
                            nc.vector.tensor_copy(oe[:, n * 512 : (n + 1) * 512], po[:])
                        else:
                            nc.scalar.copy(oe[:, n * 512 : (n + 1) * 512], po[:])
                    nc.sync.dma_start(out[i * 128 : (i + 1) * 128, :], oe[:])

                yield unit

        def interleave(primary, filler):
            """Emit primary units with filler units sprinkled between them."""
            filler = list(filler)
            primary = list(primary)
            nf, np_ = len(filler), len(primary)
            fi = 0
            for pi, p in enumerate(primary):
                # emit fillers proportionally ahead of each primary unit
                want = ((pi + 1) * nf) // (np_ + 1)
                while fi < want:
                    filler[fi]()
                    fi += 1
                p()
            while fi < nf:
                filler[fi]()
                fi += 1

        # ---- main schedule
        for u in proj_units(0):
            u()
        interleave(attn_units(0), proj_units(1))
        interleave(attn_units(1), proj_units(2))
        interleave(attn_units(2), proj_units(3))
        interleave(attn_units(3), out_units(0))
        for tcc in range(1, NTC):
            for u in out_units(tcc):
                u()

    _split_waits(nc)
    return nc


_NC_CACHE = None


def _get_nc():
    global _NC_CACHE
    if _NC_CACHE is None:
        _NC_CACHE = _build_program()
    return _NC_CACHE


def _consts():
    p = np.arange(128)
    u = np.arange(128)
    return {
        "tri": (p[:, None] <= u[None, :]),
        "ones8": np.ones((128, 8)),
        "ones64": np.ones((1, 64), dtype=np.float32),
    }


def _to_bf16(a):
    import ml_dtypes

    return np.asarray(a, dtype=np.float32).astype(ml_dtypes.bfloat16)


def make_in_maps(x, cos, sin, Wq, Wk, Wv, Wo):
    x = np.asarray(x, dtype=np.float32)
    cos = np.asarray(cos, dtype=np.float32).T  # [Dh, T]
    sin = np.asarray(sin, dtype=np.float32).T

    cos2 = np.tile(cos, (2, 1))  # [128, T]
    sin2 = np.tile(sin, (2, 1)).copy()
    sin2[0:32] *= -1.0
    sin2[64:96] *= -1.0

    c = _consts()
    consts = {
        "tri": _to_bf16(c["tri"]),
        "ones8": _to_bf16(c["ones8"]),
        "ones64": c["ones64"],
        "cos2": _to_bf16(cos2),
        "sin2": _to_bf16(sin2),
    }

    in_maps = []
    for core in range(8):
        b, hh = core // 2, core % 2
        sl = slice(hh * FC, (hh + 1) * FC)
        in_maps.append(
            {
                "xt": _to_bf16(np.ascontiguousarray(x[b].T)),
                "wq": _to_bf16(Wq[:, sl]),
                "wk": _to_bf16(Wk[:, sl]),
                "wv": _to_bf16(Wv[:, sl]),
                "wo": _to_bf16(Wo[sl, :]),
                **consts,
            }
        )
    return in_maps


def kernel(x, cos, sin, Wq, Wk, Wv, Wo):
    from concourse.bass_utils import run_bass_kernel_spmd

    in_maps = make_in_maps(x, cos, sin, Wq, Wk, Wv, Wo)
    nc = _get_nc()
    res = run_bass_kernel_spmd(nc, in_maps, core_ids=list(range(8)))
    outs = [res.results[c]["out"] for c in range(8)]
    full = np.stack([outs[2 * b] + outs[2 * b + 1] for b in range(B)])
    return full.astype(np.float32)


# revision 18
# speedup vs baseline: 1.4235x; 1.0595x over previous
"""Trainium2 Bass kernel: causal self-attention with RoPE (B=4, T=2048, D=1024, H=16, Dh=64).

Sharding: 8 cores = 4 batches x 2 head-halves. Core c handles batch c//2 and
heads (c%2)*8 .. +8 (feature columns (c%2)*512 of Wq/Wk/Wv, matching rows of
Wo). Each core computes a partial [T, D] output; the host sums the two
partials per batch.

v2 layout: x is transposed ON THE HOST to xT [D, T] bf16 (no on-chip
transposes). All matmul inputs are bf16. Single pass over all 8 heads:
  per t-chunk (512): q/k projections (transposed out [f, t]) + RoPE, v
  projection (natural [t, f] with a fused ones column per head), then
  causal attention for that chunk interleaved with the next chunk's
  projections. Scores matmuls are K=64 row-pairs (two heads concurrent in
  the PE array); AV uses M=65 (denominator fused via the ones column).
  Diagonal s-blocks narrow the exp/AV width to the causal region and apply
  a single [128,128] triangle mask. exp is the only scalar-engine work.
"""

import os
import sys

for _p in ("/opt/trn_rl_repo", "/root/.axon_site/_ro/trn_rl_repo"):
    if os.path.isdir(_p) and _p not in sys.path:
        sys.path.append(_p)

import numpy as np

import bass_rust
import concourse.bass as bass
import concourse.mybir as mybir
import concourse.tile as tile
from concourse.vector_clock import ScopedClock

F32 = mybir.dt.float32
F32R = mybir.dt.float32r
BF16 = mybir.dt.bfloat16

B, T, D, H, Dh = 4, 2048, 1024, 16, 64
FC = 512          # features per core (8 heads)
NF = 4            # head-pair tiles (128 features each)
NTC = T // 512    # 4 t-chunks
NTT = T // 128    # 16 t-tiles
ND = D // 128     # 8 d-chunks


class _TC(tile.TileContext):
    """TileContext whose tail Drain carries at most one sem wait."""

    def _drain_and_barrier(self, tick_clock, wait_clock):
        drain_inst = self.nc.sync.drain()
        wait_clock.add_sem_waits(
            drain_inst.ins, ScopedClock({None: tick_clock.global_clock})
        )
        si = drain_inst.ins.sync_info
        if si is not None and len(si.on_wait) > 1:
            waits = list(si.on_wait)
            drain_inst.ins.sync_info = bass_rust.SyncInfo(
                on_wait=waits[:1], on_update=list(si.on_update)
            )
            for w in waits[1:]:
                d2 = self.nc.sync.drain()
                d2.ins.sync_info = bass_rust.SyncInfo(on_wait=[w], on_update=[])
        self.nc.all_engine_barrier()
        popped = self.nc._tile_sem_poison_stack.pop()
        assert popped is self._sem_poison
        self.nc.clear_and_free_semaphores(list(self.sems.allocated().values()))
        self.nc.all_engine_barrier()


def _r(ap):
    return ap.bitcast(F32R)


def _split_waits(nc, max_waits=1):
    """Hoist extra sem waits onto same-engine NoOps (walrus 1-wait limit)."""
    n = 0
    for fn in nc.m.functions:
        for bb in fn.blocks:
            out = []
            for inst in bb.instructions:
                si = inst.sync_info
                if si is not None and len(si.on_wait) > max_waits:
                    waits = list(si.on_wait)
                    extra, keep = waits[:-max_waits], waits[-max_waits:]
                    for i, w in enumerate(extra):
                        nop = mybir.InstNoOp(
                            name=f"{inst.name}_ws{i}", engine=inst.engine
                        )
                        nop.sync_info = bass_rust.SyncInfo(on_wait=[w], on_update=[])
                        out.append(nop)
                        n += 1
                    inst.sync_info = bass_rust.SyncInfo(
                        on_wait=keep, on_update=list(si.on_update)
                    )
                out.append(inst)
            bb.instructions = out
    return n


def _build_program():
    from contextlib import ExitStack

    nc = bass.Bass()

    xt = nc.dram_tensor("xt", [D, T], BF16, kind="ExternalInput")
    wq = nc.dram_tensor("wq", [D, FC], BF16, kind="ExternalInput")
    wk = nc.dram_tensor("wk", [D, FC], BF16, kind="ExternalInput")
    wv = nc.dram_tensor("wv", [D, FC], BF16, kind="ExternalInput")
    wo = nc.dram_tensor("wo", [FC, D], BF16, kind="ExternalInput")
    cos2 = nc.dram_tensor("cos2", [128, T], BF16, kind="ExternalInput")
    sin2 = nc.dram_tensor("sin2", [128, T], BF16, kind="ExternalInput")
    tri = nc.dram_tensor("tri", [128, 128], BF16, kind="ExternalInput")
    ones8 = nc.dram_tensor("ones8", [128, 8], BF16, kind="ExternalInput")
    ones64 = nc.dram_tensor("ones64", [128, 64], F32R, kind="ExternalInput")
    out = nc.dram_tensor("out", [T, D], F32, kind="ExternalOutput")

    with _TC(nc) as tc, ExitStack() as ctx:
        consts = ctx.enter_context(tc.tile_pool(name="consts", bufs=1))
        persist = ctx.enter_context(tc.tile_pool(name="persist", bufs=1))
        wp = ctx.enter_context(tc.tile_pool(name="wp", bufs=1))
        # PSUM: sc 2x[128,1024] (4 banks) + ps 2x[128,512] + av 2x[128,512]
        scp = ctx.enter_context(tc.tile_pool(name="scp", bufs=2, space="PSUM"))
        psp = ctx.enter_context(tc.tile_pool(name="psp", bufs=2, space="PSUM"))
        avp = ctx.enter_context(tc.tile_pool(name="avp", bufs=2, space="PSUM"))
        # SBUF working pools
        xcp = ctx.enter_context(tc.tile_pool(name="xcp", bufs=2))
        rop = ctx.enter_context(tc.tile_pool(name="rop", bufs=2))
        exp_ = ctx.enter_context(tc.tile_pool(name="exp", bufs=4))
        nrm = ctx.enter_context(tc.tile_pool(name="nrm", bufs=3))
        oev = ctx.enter_context(tc.tile_pool(name="oev", bufs=3))

        # ---- weights / constants. Order matters: the PE's first projection
        # only needs wq + xc0, so those DMAs go first; the rest overlap the
        # first projection matmuls.
        wtiles = {}

        def load_w(nm, wsrc):
            w_t = wp.tile([128, ND * FC], BF16, tag=nm, name=f"{nm}_t")
            nc.sync.dma_start(
                w_t[:].rearrange("p (d f) -> p d f", d=ND),
                wsrc[:].rearrange("(d p) f -> p d f", p=128),
            )
            wtiles[nm] = w_t

        load_w("wq", wq)

        xc = [None] * NTC

        def emit_load_x(tcc):
            xc[tcc] = xcp.tile([128, ND * 512], BF16, tag="xc", name=f"xc{tcc}")
            nc.sync.dma_start(
                xc[tcc][:].rearrange("p (d t) -> p d t", d=ND),
                xt[:, tcc * 512 : (tcc + 1) * 512].rearrange(
                    "(d p) t -> p d t", p=128
                ),
            )

        emit_load_x(0)

        cos_t = consts.tile([128, T], BF16)
        nc.sync.dma_start(cos_t[:], cos2[:])
        sin_t = consts.tile([128, T], BF16)
        nc.sync.dma_start(sin_t[:], sin2[:])
        load_w("wk", wk)
        load_w("wv", wv)
        tri_t = consts.tile([128, 128], BF16)
        nc.sync.dma_start(tri_t[:], tri[:])
        ones64_t = consts.tile([128, 64], F32R)
        nc.sync.dma_start(ones64_t[:], ones64[:])
        wo_t = wp.tile([128, 4 * D], BF16, tag="wo", name="wo_t")
        nc.sync.dma_start(
            wo_t[:].rearrange("p (c o) -> p c o", c=4),
            wo[:].rearrange("(c p) o -> p c o", p=128),
        )

        # ---- persistent activations
        qt = [persist.tile([128, T], BF16, tag=f"qt{f}", name=f"qt{f}") for f in range(NF)]
        kt = [persist.tile([128, T], BF16, tag=f"kt{f}", name=f"kt{f}") for f in range(NF)]
        # v natural layout, 65 cols per head (ones col fused at index 64)
        vt = [persist.tile([128, 8 * 65], BF16, tag=f"vt{i}", name=f"vt{i}") for i in range(NTT)]
        ao = [persist.tile([128, T], BF16, tag=f"ao{f}", name=f"ao{f}") for f in range(NF)]
        for i in range(NTT):
            nc.vector.memset(vt[i][:, 64::65], 1.0)

        def emit_proj_mm(tcc, dst, f, rawc):
            """One q/k projection f-tile: 8 accum MMs + psum->bf16 cast."""
            w_t = wtiles["wq" if dst == 0 else "wk"]
            ps = psp.tile([128, 512], F32, tag="ps", name=f"ps{tcc}d{dst}f{f}")
            for d in range(ND):
                nc.tensor.matmul(
                    ps[:],
                    w_t[:, d * FC + f * 128 : d * FC + (f + 1) * 128],
                    xc[tcc][:, d * 512 : (d + 1) * 512],
                    start=(d == 0),
                    stop=(d == ND - 1),
                )
            nc.vector.tensor_copy(rawc[:, f * 512 : (f + 1) * 512], ps[:])

        def emit_rope(tcc, dst, rawc):
            """Batched rotate-half swap + RoPE combine for all 4 f-tiles."""
            tsl = slice(tcc * 512, (tcc + 1) * 512)
            rot = rop.tile([128, 4 * 512], BF16, tag="rot", name=f"rot{tcc}d{dst}")
            for o in (0, 64):
                nc.sync.dma_start(rot[o : o + 32, :], rawc[o + 32 : o + 64, :])
                nc.sync.dma_start(rot[o + 32 : o + 64, :], rawc[o : o + 32, :])
            for f in range(NF):
                fsl = slice(f * 512, (f + 1) * 512)
                dtile = (qt if dst == 0 else kt)[f]
                tmp = rop.tile([128, 512], BF16, tag="tmp", name=f"tmp{tcc}d{dst}f{f}")
                nc.vector.tensor_mul(tmp[:], rot[:, fsl], sin_t[:, tsl])
                nc.vector.tensor_mul(dtile[:, tsl], rawc[:, fsl], cos_t[:, tsl])
                nc.vector.tensor_add(dtile[:, tsl], dtile[:, tsl], tmp[:])

        def emit_v_unit(tcc, tb):
            """One v-projection t-tile (128 t rows, all 512 features)."""
            pv = psp.tile([128, 512], F32, tag="ps", name=f"pv{tcc}b{tb}")
            for d in range(ND):
                nc.tensor.matmul(
                    pv[:],
                    xc[tcc][:, d * 512 + tb * 128 : d * 512 + (tb + 1) * 128],
                    wtiles["wv"][:, d * FC : (d + 1) * FC],
                    start=(d == 0),
                    stop=(d == ND - 1),
                )
            i = tcc * 4 + tb
            vdst = vt[i][:, :].rearrange("p (h c) -> p h c", c=65)[:, :, 0:64]
            vsrc = pv[:].rearrange("p (h c) -> p h c", c=64)
            nc.vector.tensor_copy(vdst, vsrc)

        def proj_units(tcc, load=True):
            """Generator of emit-thunks for t-chunk tcc's projections."""
            if load:
                yield lambda: emit_load_x(tcc)
            for dst in range(2):
                rawc = [None]

                for f in range(NF):
                    def u(d=dst, ff=f, rawc=rawc):
                        if ff == 0:
                            rawc[0] = rop.tile(
                                [128, 4 * 512], BF16, tag="raw", name=f"raw{tcc}d{d}"
                            )
                        emit_proj_mm(tcc, d, ff, rawc[0])

                    yield u
                yield lambda d=dst, rawc=rawc: emit_rope(tcc, d, rawc[0])
            for tb in range(4):
                yield lambda b=tb: emit_v_unit(tcc, b)

        def attn_units(tcc, tail_state):
            """Generator of emit-thunks for attention over t-chunk tcc.

            tail_state carries the deferred norm tail across units (and
            across t-chunks); each unit emits the previous block's tail
            after its own PE work is queued.
            """
            ngrp = 2 * tcc + 2
            t0 = tcc * 512
            for fp in range(NF):
                def fp_block(fp=fp, ngrp=ngrp, t0=t0, tcc=tcc):
                    avs = []
                    for ho in range(2):
                        av = avp.tile([128, 512], F32, tag="av", name=f"av{tcc}p{fp}h{ho}")
                        avs.append(av)

                    def emit_scores(g2):
                        pair = [
                            scp.tile([128, 1024], F32, tag="sc", name=f"sc{tcc}p{fp}g{g2}h{ho}")
                            for ho in range(2)
                        ]
                        # half-outer order: (ho0,ho1) pairs hit disjoint PE
                        # row groups back-to-back and overlap in the array
                        for half in range(2):
                            si = 2 * g2 + half
                            j = si - 4 * tcc
                            col0 = j * 128 if j >= 0 else 0
                            for ho in range(2):
                                o = ho * 64
                                nc.tensor.matmul(
                                    pair[ho][:, half * 512 + col0 : (half + 1) * 512],
                                    kt[fp][o : o + 64, si * 128 : (si + 1) * 128],
                                    qt[fp][o : o + 64, t0 + col0 : t0 + 512],
                                    start=True,
                                    stop=True,
                                )
                        return pair

                    def emit_exp(g2, sc_pair):
                        diag_odd = g2 == 2 * tcc + 1
                        lo = 256 if diag_odd else 0
                        expair = []
                        for ho in range(2):
                            ex = exp_.tile([128, 1024], BF16, tag="ex", name=f"ex{tcc}p{fp}g{g2}h{ho}")
                            nc.scalar.activation(
                                ex[:, lo:1024],
                                sc_pair[ho][:, lo:1024],
                                mybir.ActivationFunctionType.Exp,
                                scale=0.125,
                            )
                            if g2 >= 2 * tcc:
                                for half in range(2):
                                    j = 2 * g2 + half - 4 * tcc
                                    c0 = half * 512 + j * 128
                                    nc.vector.tensor_mul(
                                        ex[:, c0 : c0 + 128],
                                        ex[:, c0 : c0 + 128],
                                        tri_t[:],
                                    )
                            expair.append(ex)
                        return expair

                    def emit_av(g2, expair):
                        for ho in range(2):
                            for half in range(2):
                                si = 2 * g2 + half
                                j = si - 4 * tcc
                                col0 = j * 128 if j >= 0 else 0
                                nc.tensor.matmul(
                                    avs[ho][0:65, col0:512],
                                    vt[si][:, (2 * fp + ho) * 65 : (2 * fp + ho) * 65 + 65],
                                    expair[ho][:, half * 512 + col0 : (half + 1) * 512],
                                    start=(g2 == 0 and half == 0),
                                    stop=(g2 == ngrp - 1 and half == 1),
                                )

                    # software pipeline: scores(g2) -> exp(g2) -> av(g2)
                    pend = {}
                    for g2 in range(ngrp):
                        sc_pair = emit_scores(g2)
                        pend[g2] = emit_exp(g2, sc_pair)
                        if g2 - 1 in pend:
                            emit_av(g2 - 1, pend.pop(g2 - 1))
                    emit_av(ngrp - 1, pend.pop(ngrp - 1))

                    # evict av psum + compute 1/Z now (DVE only); defer the
                    # PE broadcast + final multiply so the PE queue never
                    # waits on the reciprocal.
                    tails = []
                    for ho in range(2):
                        avsb = nrm.tile([65, 512], F32, tag="avs", name=f"avs{tcc}p{fp}h{ho}", bufs=4)
                        nc.vector.tensor_copy(avsb[:], avs[ho][0:65, :])
                        rc = nrm.tile([1, 512], F32R, tag="rc", name=f"rc{tcc}p{fp}h{ho}", bufs=4)
                        with nc.allow_low_precision(reason="f32r recip"):
                            nc.vector.reciprocal(rc[:], _r(avsb[64:65, :]))
                        tails.append((avsb, rc))

                    def norm_tail(tails=tails, fp=fp, t0=t0):
                        for ho, (avsb, rc) in enumerate(tails):
                            pb = psp.tile([128, 512], F32, tag="ps", name=f"pb{fp}h{ho}")
                            nc.tensor.matmul(
                                pb[0:64, :], ones64_t[0:1, :], rc[:], start=True, stop=True
                            )
                            nc.vector.tensor_mul(
                                ao[fp][ho * 64 : ho * 64 + 64, t0 : t0 + 512],
                                avsb[0:64, :],
                                pb[0:64, :],
                            )

                    return norm_tail

                def unit(fp_block=fp_block):
                    tail = fp_block()
                    if tail_state[0] is not None:
                        tail_state[0]()
                    tail_state[0] = tail

                yield unit

        def out_units(tcc):
            """Output projection for the 4 t-tiles of chunk tcc."""
            for tb in range(4):
                def unit(tb=tb):
                    i = tcc * 4 + tb
                    oe = oev.tile([128, D], F32, tag="oe", name=f"oe{i}")
                    for n in range(2):
                        po = psp.tile([128, 512], F32, tag="ps", name=f"po{i}n{n}")
                        for c in range(4):
                            nc.tensor.matmul(
                                po[:],
                                ao[c][:, i * 128 : (i + 1) * 128],
                                wo_t[:, c * D + n * 512 : c * D + n * 512 + 512],
                                start=(c == 0),
                                stop=(c == 3),
                            )
                        eng = nc.vector if (i + n) % 2 == 0 else nc.scalar
                        if eng is nc.vector:
                            nc.vector.tensor_copy(oe[:, n * 512 : (n + 1) * 512], po[:])
                        else:
                            nc.scalar.copy(oe[:, n * 512 : (n + 1) * 512], po[:])
                    nc.sync.dma_start(out[i * 128 : (i + 1) * 128, :], oe[:])

                yield unit

        def interleave(primary, filler):
            """Emit primary units with filler units sprinkled between them."""
            filler = list(filler)
            primary = list(primary)
            nf, np_ = len(filler), len(primary)
            fi = 0
            for pi, p in enumerate(primary):
                # emit fillers proportionally ahead of each primary unit
                want = ((pi + 1) * nf) // (np_ + 1)
                while fi < want:
                    filler[fi]()
                    fi += 1
                p()
            while fi < nf:
                filler[fi]()
                fi += 1

        # ---- main schedule
        tail_state = [None]
        for u in proj_units(0, load=False):
            u()
        interleave(attn_units(0, tail_state), proj_units(1))
        interleave(attn_units(1, tail_state), proj_units(2))
        interleave(attn_units(2, tail_state), proj_units(3))
        interleave(attn_units(3, tail_state), out_units(0))
        tail_state[0]()
        tail_state[0] = None
        for tcc in range(1, NTC):
            for u in out_units(tcc):
                u()

    _split_waits(nc)
    return nc


_NC_CACHE = None


def _get_nc():
    global _NC_CACHE
    if _NC_CACHE is None:
        _NC_CACHE = _build_program()
    return _NC_CACHE


def _consts():
    p = np.arange(128)
    u = np.arange(128)
    return {
        "tri": (p[:, None] <= u[None, :]),
        "ones8": np.ones((128, 8)),
        "ones64": np.ones((128, 64), dtype=np.float32),
    }


def _to_bf16(a):
    import ml_dtypes

    return np.asarray(a, dtype=np.float32).astype(ml_dtypes.bfloat16)


def make_in_maps(x, cos, sin, Wq, Wk, Wv, Wo):
    x = np.asarray(x, dtype=np.float32)
    cos = np.asarray(cos, dtype=np.float32).T  # [Dh, T]
    sin = np.asarray(sin, dtype=np.float32).T

    cos2 = np.tile(cos, (2, 1))  # [128, T]
    sin2 = np.tile(sin, (2, 1)).copy()
    sin2[0:32] *= -1.0
    sin2[64:96] *= -1.0

    c = _consts()
    consts = {
        "tri": _to_bf16(c["tri"]),
        "ones8": _to_bf16(c["ones8"]),
        "ones64": c["ones64"],
        "cos2": _to_bf16(cos2),
        "sin2": _to_bf16(sin2),
    }

    in_maps = []
    for core in range(8):
        b, hh = core // 2, core % 2
        sl = slice(hh * FC, (hh + 1) * FC)
        in_maps.append(
            {
                "xt": _to_bf16(np.ascontiguousarray(x[b].T)),
                "wq": _to_bf16(Wq[:, sl]),
                "wk": _to_bf16(Wk[:, sl]),
                "wv": _to_bf16(Wv[:, sl]),
                "wo": _to_bf16(Wo[sl, :]),
                **consts,
            }
        )
    return in_maps


def kernel(x, cos, sin, Wq, Wk, Wv, Wo):
    from concourse.bass_utils import run_bass_kernel_spmd

    in_maps = make_in_maps(x, cos, sin, Wq, Wk, Wv, Wo)
    nc = _get_nc()
    res = run_bass_kernel_spmd(nc, in_maps, core_ids=list(range(8)))
    outs = [res.results[c]["out"] for c in range(8)]
    full = np.stack([outs[2 * b] + outs[2 * b + 1] for b in range(B)])
    return full.astype(np.float32)


# revision 19
# speedup vs baseline: 1.4683x; 1.0315x over previous
"""Trainium2 Bass kernel: causal self-attention with RoPE (B=4, T=2048, D=1024, H=16, Dh=64).

Sharding: 8 cores = 4 batches x 2 head-halves. Core c handles batch c//2 and
heads (c%2)*8 .. +8 (feature columns (c%2)*512 of Wq/Wk/Wv, matching rows of
Wo). Each core computes a partial [T, D] output; the host sums the two
partials per batch.

v2 layout: x is transposed ON THE HOST to xT [D, T] bf16 (no on-chip
transposes). All matmul inputs are bf16. Single pass over all 8 heads:
  per t-chunk (512): q/k projections (transposed out [f, t]) + RoPE, v
  projection (natural [t, f] with a fused ones column per head), then
  causal attention for that chunk interleaved with the next chunk's
  projections. Scores matmuls are K=64 row-pairs (two heads concurrent in
  the PE array); AV uses M=65 (denominator fused via the ones column).
  Diagonal s-blocks narrow the exp/AV width to the causal region and apply
  a single [128,128] triangle mask. exp is the only scalar-engine work.
"""

import os
import sys

for _p in ("/opt/trn_rl_repo", "/root/.axon_site/_ro/trn_rl_repo"):
    if os.path.isdir(_p) and _p not in sys.path:
        sys.path.append(_p)

import numpy as np

import bass_rust
import concourse.bass as bass
import concourse.mybir as mybir
import concourse.tile as tile
from concourse.vector_clock import ScopedClock

F32 = mybir.dt.float32
F32R = mybir.dt.float32r
BF16 = mybir.dt.bfloat16

B, T, D, H, Dh = 4, 2048, 1024, 16, 64
FC = 512          # features per core (8 heads)
NF = 4            # head-pair tiles (128 features each)
NTC = T // 512    # 4 t-chunks
NTT = T // 128    # 16 t-tiles
ND = D // 128     # 8 d-chunks


class _TC(tile.TileContext):
    """TileContext whose tail Drain carries at most one sem wait."""

    def _drain_and_barrier(self, tick_clock, wait_clock):
        drain_inst = self.nc.sync.drain()
        wait_clock.add_sem_waits(
            drain_inst.ins, ScopedClock({None: tick_clock.global_clock})
        )
        si = drain_inst.ins.sync_info
        if si is not None and len(si.on_wait) > 1:
            waits = list(si.on_wait)
            drain_inst.ins.sync_info = bass_rust.SyncInfo(
                on_wait=waits[:1], on_update=list(si.on_update)
            )
            for w in waits[1:]:
                d2 = self.nc.sync.drain()
                d2.ins.sync_info = bass_rust.SyncInfo(on_wait=[w], on_update=[])
        self.nc.all_engine_barrier()
        popped = self.nc._tile_sem_poison_stack.pop()
        assert popped is self._sem_poison
        self.nc.clear_and_free_semaphores(list(self.sems.allocated().values()))
        self.nc.all_engine_barrier()


def _r(ap):
    return ap.bitcast(F32R)


def _split_waits(nc, max_waits=1):
    """Hoist extra sem waits onto same-engine NoOps (walrus 1-wait limit)."""
    n = 0
    for fn in nc.m.functions:
        for bb in fn.blocks:
            out = []
            for inst in bb.instructions:
                si = inst.sync_info
                if si is not None and len(si.on_wait) > max_waits:
                    waits = list(si.on_wait)
                    extra, keep = waits[:-max_waits], waits[-max_waits:]
                    for i, w in enumerate(extra):
                        nop = mybir.InstNoOp(
                            name=f"{inst.name}_ws{i}", engine=inst.engine
                        )
                        nop.sync_info = bass_rust.SyncInfo(on_wait=[w], on_update=[])
                        out.append(nop)
                        n += 1
                    inst.sync_info = bass_rust.SyncInfo(
                        on_wait=keep, on_update=list(si.on_update)
                    )
                out.append(inst)
            bb.instructions = out
    return n


def _build_program():
    from contextlib import ExitStack

    nc = bass.Bass()

    xt = nc.dram_tensor("xt", [D, T], BF16, kind="ExternalInput")
    wq = nc.dram_tensor("wq", [D, FC], BF16, kind="ExternalInput")
    wk = nc.dram_tensor("wk", [D, FC], BF16, kind="ExternalInput")
    wv = nc.dram_tensor("wv", [D, FC], BF16, kind="ExternalInput")
    wo = nc.dram_tensor("wo", [FC, D], BF16, kind="ExternalInput")
    cos2 = nc.dram_tensor("cos2", [128, T], BF16, kind="ExternalInput")
    sin2 = nc.dram_tensor("sin2", [128, T], BF16, kind="ExternalInput")
    tri = nc.dram_tensor("tri", [128, 128], BF16, kind="ExternalInput")
    ones8 = nc.dram_tensor("ones8", [128, 8], BF16, kind="ExternalInput")
    ones64 = nc.dram_tensor("ones64", [128, 64], F32R, kind="ExternalInput")
    out = nc.dram_tensor("out", [T, D], F32, kind="ExternalOutput")

    with _TC(nc) as tc, ExitStack() as ctx:
        consts = ctx.enter_context(tc.tile_pool(name="consts", bufs=1))
        persist = ctx.enter_context(tc.tile_pool(name="persist", bufs=1))
        wp = ctx.enter_context(tc.tile_pool(name="wp", bufs=1))
        # PSUM: sc 2x[128,1024] (4 banks) + ps 2x[128,512] + av 2x[128,512]
        scp = ctx.enter_context(tc.tile_pool(name="scp", bufs=2, space="PSUM"))
        psp = ctx.enter_context(tc.tile_pool(name="psp", bufs=2, space="PSUM"))
        avp = ctx.enter_context(tc.tile_pool(name="avp", bufs=2, space="PSUM"))
        # SBUF working pools
        xcp = ctx.enter_context(tc.tile_pool(name="xcp", bufs=2))
        rop = ctx.enter_context(tc.tile_pool(name="rop", bufs=2))
        exp_ = ctx.enter_context(tc.tile_pool(name="exp", bufs=4))
        nrm = ctx.enter_context(tc.tile_pool(name="nrm", bufs=3))
        oev = ctx.enter_context(tc.tile_pool(name="oev", bufs=3))

        # ---- weights / constants. Order matters: the PE's first projection
        # only needs wq + xc0, so those DMAs go first; the rest overlap the
        # first projection matmuls.
        wtiles = {}

        def load_w(nm, wsrc):
            w_t = wp.tile([128, ND * FC], BF16, tag=nm, name=f"{nm}_t")
            nc.sync.dma_start(
                w_t[:].rearrange("p (d f) -> p d f", d=ND),
                wsrc[:].rearrange("(d p) f -> p d f", p=128),
            )
            wtiles[nm] = w_t

        load_w("wq", wq)

        xc = [None] * NTC

        def emit_load_x(tcc):
            xc[tcc] = xcp.tile([128, ND * 512], BF16, tag="xc", name=f"xc{tcc}")
            nc.sync.dma_start(
                xc[tcc][:].rearrange("p (d t) -> p d t", d=ND),
                xt[:, tcc * 512 : (tcc + 1) * 512].rearrange(
                    "(d p) t -> p d t", p=128
                ),
            )

        emit_load_x(0)

        cos_t = consts.tile([128, T], BF16)
        nc.sync.dma_start(cos_t[:], cos2[:])
        sin_t = consts.tile([128, T], BF16)
        nc.sync.dma_start(sin_t[:], sin2[:])
        load_w("wk", wk)
        load_w("wv", wv)
        tri_t = consts.tile([128, 128], BF16)
        nc.sync.dma_start(tri_t[:], tri[:])
        ones64_t = consts.tile([128, 64], F32R)
        nc.sync.dma_start(ones64_t[:], ones64[:])
        wo_t = wp.tile([128, 4 * D], BF16, tag="wo", name="wo_t")
        nc.sync.dma_start(
            wo_t[:].rearrange("p (c o) -> p c o", c=4),
            wo[:].rearrange("(c p) o -> p c o", p=128),
        )

        # ---- persistent activations
        qt = [persist.tile([128, T], BF16, tag=f"qt{f}", name=f"qt{f}") for f in range(NF)]
        kt = [persist.tile([128, T], BF16, tag=f"kt{f}", name=f"kt{f}") for f in range(NF)]
        # v natural layout, 65 cols per head (ones col fused at index 64)
        vt = [persist.tile([128, 8 * 65], BF16, tag=f"vt{i}", name=f"vt{i}") for i in range(NTT)]
        ao = [persist.tile([128, T], BF16, tag=f"ao{f}", name=f"ao{f}") for f in range(NF)]
        for i in range(NTT):
            nc.vector.memset(vt[i][:, 64::65], 1.0)

        def emit_proj_mm(tcc, dst, f, rawc):
            """One q/k projection f-tile: 8 accum MMs + psum->bf16 cast."""
            w_t = wtiles["wq" if dst == 0 else "wk"]
            ps = psp.tile([128, 512], F32, tag="ps", name=f"ps{tcc}d{dst}f{f}")
            for d in range(ND):
                nc.tensor.matmul(
                    ps[:],
                    w_t[:, d * FC + f * 128 : d * FC + (f + 1) * 128],
                    xc[tcc][:, d * 512 : (d + 1) * 512],
                    start=(d == 0),
                    stop=(d == ND - 1),
                )
            nc.vector.tensor_copy(rawc[:, f * 512 : (f + 1) * 512], ps[:])

        def emit_rope(tcc, dst, rawc):
            """Batched rotate-half swap + RoPE combine for all 4 f-tiles."""
            tsl = slice(tcc * 512, (tcc + 1) * 512)
            rot = rop.tile([128, 4 * 512], BF16, tag="rot", name=f"rot{tcc}d{dst}")
            for o in (0, 64):
                nc.sync.dma_start(rot[o : o + 32, :], rawc[o + 32 : o + 64, :])
                nc.sync.dma_start(rot[o + 32 : o + 64, :], rawc[o : o + 32, :])
            for f in range(NF):
                fsl = slice(f * 512, (f + 1) * 512)
                dtile = (qt if dst == 0 else kt)[f]
                tmp = rop.tile([128, 512], BF16, tag="tmp", name=f"tmp{tcc}d{dst}f{f}")
                nc.vector.tensor_mul(tmp[:], rot[:, fsl], sin_t[:, tsl])
                nc.vector.tensor_mul(dtile[:, tsl], rawc[:, fsl], cos_t[:, tsl])
                nc.vector.tensor_add(dtile[:, tsl], dtile[:, tsl], tmp[:])

        def emit_v_unit(tcc, tb):
            """One v-projection t-tile (128 t rows, all 512 features)."""
            pv = psp.tile([128, 512], F32, tag="ps", name=f"pv{tcc}b{tb}")
            for d in range(ND):
                nc.tensor.matmul(
                    pv[:],
                    xc[tcc][:, d * 512 + tb * 128 : d * 512 + (tb + 1) * 128],
                    wtiles["wv"][:, d * FC : (d + 1) * FC],
                    start=(d == 0),
                    stop=(d == ND - 1),
                )
            i = tcc * 4 + tb
            vdst = vt[i][:, :].rearrange("p (h c) -> p h c", c=65)[:, :, 0:64]
            vsrc = pv[:].rearrange("p (h c) -> p h c", c=64)
            nc.vector.tensor_copy(vdst, vsrc)

        def proj_units(tcc, load=True):
            """Generator of emit-thunks for t-chunk tcc's projections."""
            if load:
                yield lambda: emit_load_x(tcc)
            for dst in range(2):
                rawc = [None]

                for f in range(NF):
                    def u(d=dst, ff=f, rawc=rawc):
                        if ff == 0:
                            rawc[0] = rop.tile(
                                [128, 4 * 512], BF16, tag="raw", name=f"raw{tcc}d{d}"
                            )
                        emit_proj_mm(tcc, d, ff, rawc[0])

                    yield u
                yield lambda d=dst, rawc=rawc: emit_rope(tcc, d, rawc[0])
            for tb in range(4):
                yield lambda b=tb: emit_v_unit(tcc, b)

        def attn_units(tcc, tail_state, fillers):
            """Generator of emit-thunks for attention over t-chunk tcc.

            tail_state carries the deferred norm tail across units (and
            across t-chunks); each unit emits the previous block's tail
            after its own PE work is queued.
            """
            ngrp = 2 * tcc + 2
            t0 = tcc * 512
            for fp in range(NF):
                def fp_block(fp=fp, ngrp=ngrp, t0=t0, tcc=tcc):
                    avs = []
                    for ho in range(2):
                        av = avp.tile([128, 512], F32, tag="av", name=f"av{tcc}p{fp}h{ho}")
                        avs.append(av)

                    def emit_scores(g2):
                        pair = [
                            scp.tile([128, 1024], F32, tag="sc", name=f"sc{tcc}p{fp}g{g2}h{ho}")
                            for ho in range(2)
                        ]
                        # half-outer order: (ho0,ho1) pairs hit disjoint PE
                        # row groups back-to-back and overlap in the array
                        for half in range(2):
                            si = 2 * g2 + half
                            j = si - 4 * tcc
                            col0 = j * 128 if j >= 0 else 0
                            for ho in range(2):
                                o = ho * 64
                                nc.tensor.matmul(
                                    pair[ho][:, half * 512 + col0 : (half + 1) * 512],
                                    kt[fp][o : o + 64, si * 128 : (si + 1) * 128],
                                    qt[fp][o : o + 64, t0 + col0 : t0 + 512],
                                    start=True,
                                    stop=True,
                                )
                        return pair

                    def emit_exp(g2, sc_pair):
                        diag_odd = g2 == 2 * tcc + 1
                        lo = 256 if diag_odd else 0
                        expair = []
                        for ho in range(2):
                            ex = exp_.tile([128, 1024], BF16, tag="ex", name=f"ex{tcc}p{fp}g{g2}h{ho}")
                            nc.scalar.activation(
                                ex[:, lo:1024],
                                sc_pair[ho][:, lo:1024],
                                mybir.ActivationFunctionType.Exp,
                                scale=0.125,
                            )
                            if g2 >= 2 * tcc:
                                for half in range(2):
                                    j = 2 * g2 + half - 4 * tcc
                                    c0 = half * 512 + j * 128
                                    nc.vector.tensor_mul(
                                        ex[:, c0 : c0 + 128],
                                        ex[:, c0 : c0 + 128],
                                        tri_t[:],
                                    )
                            expair.append(ex)
                        return expair

                    def emit_av(g2, expair):
                        for ho in range(2):
                            for half in range(2):
                                si = 2 * g2 + half
                                j = si - 4 * tcc
                                col0 = j * 128 if j >= 0 else 0
                                nc.tensor.matmul(
                                    avs[ho][0:65, col0:512],
                                    vt[si][:, (2 * fp + ho) * 65 : (2 * fp + ho) * 65 + 65],
                                    expair[ho][:, half * 512 + col0 : (half + 1) * 512],
                                    start=(g2 == 0 and half == 0),
                                    stop=(g2 == ngrp - 1 and half == 1),
                                )

                    # software pipeline: scores(g2) -> exp(g2) -> av(g2).
                    # A filler (projection/out-proj unit) is emitted right
                    # after the first scores pair so the PE has work while
                    # the first exp of the block is still running.
                    pend = {}
                    for g2 in range(ngrp):
                        sc_pair = emit_scores(g2)
                        pend[g2] = emit_exp(g2, sc_pair)
                        if g2 == 0 and fillers:
                            fillers.popleft()()
                        if g2 - 1 in pend:
                            emit_av(g2 - 1, pend.pop(g2 - 1))
                    emit_av(ngrp - 1, pend.pop(ngrp - 1))

                    # evict av psum + compute 1/Z now (DVE only); defer the
                    # PE broadcast + final multiply so the PE queue never
                    # waits on the reciprocal.
                    tails = []
                    for ho in range(2):
                        avsb = nrm.tile([65, 512], F32, tag="avs", name=f"avs{tcc}p{fp}h{ho}", bufs=4)
                        nc.vector.tensor_copy(avsb[:], avs[ho][0:65, :])
                        rc = nrm.tile([1, 512], F32R, tag="rc", name=f"rc{tcc}p{fp}h{ho}", bufs=4)
                        with nc.allow_low_precision(reason="f32r recip"):
                            nc.vector.reciprocal(rc[:], _r(avsb[64:65, :]))
                        tails.append((avsb, rc))

                    def norm_tail(tails=tails, fp=fp, t0=t0):
                        for ho, (avsb, rc) in enumerate(tails):
                            pb = psp.tile([128, 512], F32, tag="ps", name=f"pb{fp}h{ho}")
                            nc.tensor.matmul(
                                pb[0:64, :], ones64_t[0:1, :], rc[:], start=True, stop=True
                            )
                            nc.vector.tensor_mul(
                                ao[fp][ho * 64 : ho * 64 + 64, t0 : t0 + 512],
                                avsb[0:64, :],
                                pb[0:64, :],
                            )

                    return norm_tail

                def unit(fp_block=fp_block):
                    if fillers:
                        fillers.popleft()()
                    tail = fp_block()
                    if tail_state[0] is not None:
                        tail_state[0]()
                    tail_state[0] = tail

                yield unit

        def out_units(tcc):
            """Output projection for the 4 t-tiles of chunk tcc."""
            for tb in range(4):
                def unit(tb=tb):
                    i = tcc * 4 + tb
                    oe = oev.tile([128, D], F32, tag="oe", name=f"oe{i}")
                    for n in range(2):
                        po = psp.tile([128, 512], F32, tag="ps", name=f"po{i}n{n}")
                        for c in range(4):
                            nc.tensor.matmul(
                                po[:],
                                ao[c][:, i * 128 : (i + 1) * 128],
                                wo_t[:, c * D + n * 512 : c * D + n * 512 + 512],
                                start=(c == 0),
                                stop=(c == 3),
                            )
                        eng = nc.vector if (i + n) % 2 == 0 else nc.scalar
                        if eng is nc.vector:
                            nc.vector.tensor_copy(oe[:, n * 512 : (n + 1) * 512], po[:])
                        else:
                            nc.scalar.copy(oe[:, n * 512 : (n + 1) * 512], po[:])
                    nc.sync.dma_start(out[i * 128 : (i + 1) * 128, :], oe[:])

                yield unit

        from collections import deque

        def interleave(primary, filler, fillers):
            """Emit primary units; filler units are drained by the
            primaries themselves at PE-stall points (plus any leftovers)."""
            fillers.extend(filler)
            for p in primary:
                p()
            while fillers:
                fillers.popleft()()

        # ---- main schedule
        tail_state = [None]
        fillers = deque()
        for u in proj_units(0, load=False):
            u()
        interleave(attn_units(0, tail_state, fillers), proj_units(1), fillers)
        interleave(attn_units(1, tail_state, fillers), proj_units(2), fillers)
        interleave(attn_units(2, tail_state, fillers), proj_units(3), fillers)
        interleave(attn_units(3, tail_state, fillers), out_units(0), fillers)
        tail_state[0]()
        tail_state[0] = None
        for tcc in range(1, NTC):
            for u in out_units(tcc):
                u()

    _split_waits(nc)
    return nc


_NC_CACHE = None


def _get_nc():
    global _NC_CACHE
    if _NC_CACHE is None:
        _NC_CACHE = _build_program()
    return _NC_CACHE


def _consts():
    p = np.arange(128)
    u = np.arange(128)
    return {
        "tri": (p[:, None] <= u[None, :]),
        "ones8": np.ones((128, 8)),
        "ones64": np.ones((128, 64), dtype=np.float32),
    }


def _to_bf16(a):
    import ml_dtypes

    return np.asarray(a, dtype=np.float32).astype(ml_dtypes.bfloat16)


def make_in_maps(x, cos, sin, Wq, Wk, Wv, Wo):
    x = np.asarray(x, dtype=np.float32)
    cos = np.asarray(cos, dtype=np.float32).T  # [Dh, T]
    sin = np.asarray(sin, dtype=np.float32).T

    cos2 = np.tile(cos, (2, 1))  # [128, T]
    sin2 = np.tile(sin, (2, 1)).copy()
    sin2[0:32] *= -1.0
    sin2[64:96] *= -1.0

    c = _consts()
    consts = {
        "tri": _to_bf16(c["tri"]),
        "ones8": _to_bf16(c["ones8"]),
        "ones64": c["ones64"],
        "cos2": _to_bf16(cos2),
        "sin2": _to_bf16(sin2),
    }

    in_maps = []
    for core in range(8):
        b, hh = core // 2, core % 2
        sl = slice(hh * FC, (hh + 1) * FC)
        in_maps.append(
            {
                "xt": _to_bf16(np.ascontiguousarray(x[b].T)),
                "wq": _to_bf16(Wq[:, sl]),
                "wk": _to_bf16(Wk[:, sl]),
                "wv": _to_bf16(Wv[:, sl]),
                "wo": _to_bf16(Wo[sl, :]),
                **consts,
            }
        )
    return in_maps


def kernel(x, cos, sin, Wq, Wk, Wv, Wo):
    from concourse.bass_utils import run_bass_kernel_spmd

    in_maps = make_in_maps(x, cos, sin, Wq, Wk, Wv, Wo)
    nc = _get_nc()
    res = run_bass_kernel_spmd(nc, in_maps, core_ids=list(range(8)))
    outs = [res.results[c]["out"] for c in range(8)]
    full = np.stack([outs[2 * b] + outs[2 * b + 1] for b in range(B)])
    return full.astype(np.float32)
